# revision 31
# baseline (speedup 1.0000x reference)
"""Causal self-attention (dense transformer block) on 8 TRN2 NeuronCores.

Tensor-parallel over heads: 16 heads / 8 cores -> 2 heads per core, both
batch elements on every core. Per core:
  - QKV projection in "T layout": q^T/k^T per head [dh, tok] (bias fused
    into the PSUM eviction), V natural [tok, dh] for both heads
  - causal attention with scores in transposed layout [k, q], q-chunk
    outer loop (QC=512):
      * scores matmul streams only the causally-live columns (no memsets)
      * softmax numerator via ACT exp (scale folded)
      * denominators accumulate on the PE: ones^T @ P into a PSUM row,
        start/stop spanning the k loop -- no DVE row-sum adds at all
      * unnormalized attn output accumulates in PSUM in [dh, q] layout
      * 1/denominator is broadcast to all partitions with a K=1 matmul
        and applied by DVE at PSUM eviction, so `us` is already
        normalized
  - out_proj per chunk: both heads accumulate into ONE PSUM bank; a
    single DVE tensor_add evicts it and adds the folded bias (v-bias
    term and b_out pre-combined on host)
  - core returns a partial [2, 2048, 2048]; host sums the 8 partials
Matmuls run as float32r (TF32-like, full PE rate at free dim >= 256).
"""
import sys

if "/opt/trn_rl_repo" not in sys.path:
    sys.path.insert(0, "/opt/trn_rl_repo")

import numpy as np

import concourse.bacc as bacc
import concourse.bass as bass
import concourse.mybir as mybir
import concourse.tile as tile
from concourse.bass_utils import run_bass_kernel_spmd

P = 128
B, S, D = 2, 2048, 2048
H, DH = 16, 128
HPC = 2            # heads per core
NCORES = 8
TC = 256           # token chunk for the QKV projection
QC = 512           # q chunk for attention
NQC = S // QC      # 4 q chunks
KPQ = QC // P      # 4 key blocks per q chunk
SCALE = 1.0 / float(np.sqrt(DH))

f32 = mybir.dt.float32
f32r = mybir.dt.float32r
Act = mybir.ActivationFunctionType


def _emit(nc, tc_ctx, aps):
    xty, wqkv, bqk, wout, brep, trilm, ones, out_p = aps
    tc = tc_ctx
    NTB = S // P            # 16 token blocks per batch
    NDC = D // P            # 16 contraction chunks

    with (
        tc.tile_pool(name="const", bufs=1) as const,
        tc.tile_pool(name="xtp", bufs=2) as xtp,
        tc.tile_pool(name="qk", bufs=1) as qk,
        tc.tile_pool(name="vp", bufs=1) as vp,
        tc.tile_pool(name="pp", bufs=4) as pp,
        tc.tile_pool(name="up", bufs=1) as up,
        tc.tile_pool(name="rr", bufs=2) as rr,
        tc.tile_pool(name="rb", bufs=2) as rb,
        tc.tile_pool(name="fin", bufs=4) as fin,
        tc.tile_pool(name="ps_p", bufs=2, space="PSUM") as ps_p,
        tc.tile_pool(name="ps_o", bufs=2, space="PSUM") as ps_o,
        tc.tile_pool(name="ps_u", bufs=2, space="PSUM") as ps_u,
        tc.tile_pool(name="ps_d", bufs=2, space="PSUM") as ps_d,
    ):
        bqk_sb = const.tile([P, 4], f32)
        nc.sync.dma_start(bqk_sb, bqk)
        w_sb = const.tile([P, NDC, 6 * P], f32r)
        wqkv_r = wqkv.rearrange("(dc p) c -> p dc c", p=P).bitcast(f32r)
        for dc in range(NDC):
            nc.sync.dma_start(w_sb[:, dc, :], wqkv_r[:, dc, :])
        tril_sb = const.tile([P, P], f32)
        ones_sb = const.tile([P, 1], f32r)
        onesrow = const.tile([1, P], f32)
        brep_sb = const.tile([P, D], f32)
        wo_sb = const.tile([P, HPC, D], f32r)

        def load_attn_consts():
            nc.sync.dma_start(tril_sb, trilm)
            nc.sync.dma_start(ones_sb, ones.bitcast(f32r))
            nc.sync.dma_start(onesrow, ones.rearrange("p o -> o p"))

        def load_oproj_consts():
            nc.sync.dma_start(
                wo_sb, wout.rearrange("(h p) c -> p h c", p=P).bitcast(f32r)
            )
            nc.sync.dma_start(brep_sb, brep)

        # out-proj tiles are deferred and drip-fed into later loops so the
        # PE queue never blocks on the normalization chain
        pending = []

        def emit_outproj(n):
            for _ in range(min(n, len(pending))):
                b_, tb, cc, us_ = pending.pop(0)
                pso = ps_o.tile([P, QC], f32, tag="o", name="pso")
                nc.tensor.matmul(
                    pso,
                    us_[0][:, tb * P:(tb + 1) * P],
                    wo_sb[:, 0, cc * QC:(cc + 1) * QC],
                    start=True,
                    stop=False,
                )
                nc.tensor.matmul(
                    pso,
                    us_[1][:, tb * P:(tb + 1) * P],
                    wo_sb[:, 1, cc * QC:(cc + 1) * QC],
                    start=False,
                    stop=True,
                )
                f_t = fin.tile([P, QC], f32, tag="fin", name="f_t")
                nc.vector.tensor_add(
                    f_t, pso, brep_sb[:, cc * QC:(cc + 1) * QC]
                )
                nc.sync.dma_start(
                    out_p[b_, tb * P:(tb + 1) * P, cc * QC:(cc + 1) * QC],
                    f_t,
                )

        for b in range(B):
            # ---------------- QKV projection ----------------
            q_sb = [qk.tile([P, S], f32r, tag=f"q{h}", name=f"q{h}") for h in range(HPC)]
            k_sb = [qk.tile([P, S], f32r, tag=f"k{h}", name=f"k{h}") for h in range(HPC)]
            v_sb = vp.tile([P, NTB, HPC * DH], f32r, tag="v", name="v_sb")

            for tci in range(S // TC):
                xt = xtp.tile([P, NDC, TC], f32r, tag="xt", name="xt")
                xsrc = (
                    xty[b, :, tci * TC:(tci + 1) * TC]
                    .rearrange("(dc p) t -> p dc t", p=P)
                    .bitcast(f32r)
                )
                for g in range(4):
                    nc.sync.dma_start(
                        xt[:, 4 * g:4 * g + 4, :], xsrc[:, 4 * g:4 * g + 4, :]
                    )
                # q^T / k^T for both heads: psum [col=128, tok=TC]
                for cb in range(4):
                    psq = (ps_p if cb % 2 else ps_o).tile(
                        [P, TC], f32, tag="o" if cb % 2 == 0 else "w", name="psq"
                    )
                    for dc in range(NDC):
                        nc.tensor.matmul(
                            psq,
                            w_sb[:, dc, cb * P:(cb + 1) * P],
                            xt[:, dc, :],
                            start=(dc == 0),
                            stop=(dc == NDC - 1),
                        )
                    dst = q_sb[cb] if cb < HPC else k_sb[cb - HPC]
                    nc.scalar.activation(
                        dst[:, tci * TC:(tci + 1) * TC],
                        psq,
                        Act.Identity,
                        bias=bqk_sb[:, cb:cb + 1],
                    )
                # V natural for both heads: psum [tok=128, 2*dh]
                for tb in range(TC // P):
                    psv = ps_u.tile([P, HPC * DH], f32, tag="u", name="psv")
                    for dc in range(NDC):
                        nc.tensor.matmul(
                            psv,
                            xt[:, dc, tb * P:(tb + 1) * P],
                            w_sb[:, dc, 4 * P:6 * P],
                            start=(dc == 0),
                            stop=(dc == NDC - 1),
                        )
                    nc.scalar.copy(v_sb[:, tci * (TC // P) + tb, :], psv)
                if b == 0 and tci == 2:
                    load_attn_consts()
                if b == 0 and tci == 4:
                    load_oproj_consts()
                emit_outproj(2)

            # ---------------- attention, q-chunk outer ----------------
            us = [
                up.tile([P, S], f32r, tag=f"u{h}", name=f"u{h}")
                for h in range(HPC)
            ]
            for c in range(NQC):
                nkb = KPQ * c + KPQ          # key blocks 0 .. 4c+3
                psu = [
                    ps_u.tile([P, QC], f32, tag="u", name=f"psu{h}")
                    for h in range(HPC)
                ]
                dp = [
                    ps_d.tile([P, QC], f32, tag="d", name=f"dp{h}")
                    for h in range(HPC)
                ]
                nunits = max((nkb - 2) * HPC, 1)
                drip = (len(pending) + nunits - 1) // nunits

                # one-deep software pipeline: the PV and denominator matmuls
                # of unit i are emitted AFTER the scores matmul of unit i+1,
                # so the PE always has independent work queued while ACT runs
                # the exp that PV depends on
                def emit_pv(u):
                    kb_, h_, off_, pt_ = u
                    nc.tensor.matmul(
                        psu[h_][:, off_:],
                        v_sb[:, kb_, h_ * DH:(h_ + 1) * DH],
                        pt_[:, off_:],
                        start=(kb_ == 0),
                        stop=(kb_ == nkb - 1),
                    )
                    nc.tensor.matmul(
                        dp[h_][0:1, off_:],
                        ones_sb,
                        pt_[:, off_:],
                        start=(kb_ == 0),
                        stop=(kb_ == nkb - 1),
                    )

                inflight = []
                for kb in range(nkb):
                    off = max(0, (kb - KPQ * c) * P)
                    for h in range(HPC):
                        psp = ps_p.tile([P, QC], f32, tag="w", name="psp")
                        nc.tensor.matmul(
                            psp[:, off:],
                            k_sb[h][:, kb * P:(kb + 1) * P],
                            q_sb[h][:, c * QC + off:(c + 1) * QC],
                            start=True,
                            stop=True,
                        )
                        if kb >= KPQ * c:
                            nc.vector.tensor_add(
                                psp[:, off:off + P],
                                psp[:, off:off + P],
                                tril_sb,
                            )
                        p_t = pp.tile([P, QC], f32r, tag="p", name="p_t")
                        nc.scalar.activation(
                            p_t[:, off:], psp[:, off:], Act.Exp, scale=SCALE
                        )
                        # out-proj drip sits between the scores matmul and
                        # the exp-dependent PV pair: >1us of independent PE
                        # work covers the ACT latency
                        if kb >= 2:
                            emit_outproj(drip)
                        if len(inflight) == 1:
                            emit_pv(inflight.pop(0))
                        inflight.append((kb, h, off, p_t))
                for u in inflight:
                    emit_pv(u)
                emit_outproj(len(pending))
                # normalize + evict each head's chunk
                rrow = []
                for h in range(HPC):
                    # denominators are well away from 0/inf, so the 18-bit
                    # reciprocal is plenty; 5x faster than reciprocal()
                    r = rr.tile([1, QC], f32, tag=f"r{h}", name="rrow")
                    nc.vector.reciprocal_approx_fast(r, dp[h][0:1, :])
                    rrow.append(r)
                rbc_ps = []
                for h in range(HPC):
                    rp = ps_o.tile([P, QC], f32, tag="o", name="rbc_ps")
                    nc.tensor.matmul(rp, onesrow, rrow[h], start=True, stop=True)
                    rbc_ps.append(rp)
                for h in range(HPC):
                    rbc = rb.tile([P, QC], f32, tag=f"rbc{h}", name="rbc")
                    nc.vector.tensor_copy(out=rbc, in_=rbc_ps[h])
                    nc.vector.tensor_mul(
                        us[h][:, c * QC:(c + 1) * QC],
                        psu[h],
                        rbc,
                    )
                # defer this chunk's out-proj tiles
                for tb in range(KPQ * c, KPQ * c + KPQ):
                    for cc in range(D // QC):
                        pending.append((b, tb, cc, us))
        emit_outproj(len(pending))


_CACHE = {}


def _build():
    if "nc" in _CACHE:
        return _CACHE["nc"]
    nc = bacc.Bacc("TRN2", target_bir_lowering=False, debug=False)
    xty = nc.dram_tensor("xty", [B, D, S], f32, kind="ExternalInput").ap()
    wqkv = nc.dram_tensor("wqkv", [D, 6 * P], f32, kind="ExternalInput").ap()
    bqk = nc.dram_tensor("bqk", [P, 4], f32, kind="ExternalInput").ap()
    wout = nc.dram_tensor("wout", [HPC * DH, D], f32, kind="ExternalInput").ap()
    brep = nc.dram_tensor("brep", [P, D], f32, kind="ExternalInput").ap()
    trilm = nc.dram_tensor("trilm", [P, P], f32, kind="ExternalInput").ap()
    ones = nc.dram_tensor("ones", [P, 1], f32, kind="ExternalInput").ap()
    out_p = nc.dram_tensor("out_p", [B, S, D], f32, kind="ExternalOutput").ap()
    with tile.TileContext(nc) as tctx:
        _emit(nc, tctx, (xty, wqkv, bqk, wout, brep, trilm, ones, out_p))
    nc.compile()
    _CACHE["nc"] = nc
    return nc


def _in_maps(x, W_qkv, b_qkv, W_out, b_out):
    trilm = np.where(
        np.arange(P)[None, :] >= np.arange(P)[:, None], 0.0, -1e9
    ).astype(np.float32)
    ones = np.ones((P, 1), dtype=np.float32)
    xty = np.ascontiguousarray(x.transpose(0, 2, 1))
    maps = []
    for core in range(NCORES):
        h0 = core * HPC
        cols = []
        for off in (0, D, 2 * D):  # q, k, v column groups of W_qkv
            for h in range(h0, h0 + HPC):
                cols.append((off + h * DH, off + (h + 1) * DH))
        wqkv_c = np.concatenate(
            [W_qkv[:, a:b_] for a, b_ in cols], axis=1
        ).astype(np.float32)
        bqk_c = np.stack(
            [b_qkv[a:b_] for a, b_ in cols[:4]], axis=1
        ).astype(np.float32)  # [128, 4]
        bv_c = np.concatenate(
            [b_qkv[a:b_] for a, b_ in cols[4:]]
        ).astype(np.float32)  # [256]
        wout_c = W_out[h0 * DH:(h0 + HPC) * DH, :].astype(np.float32)
        bias_fold = (b_out / NCORES + bv_c @ wout_c).astype(np.float32)
        brep_c = np.broadcast_to(bias_fold, (P, D)).copy()
        maps.append({
            "xty": xty,
            "wqkv": np.ascontiguousarray(wqkv_c),
            "bqk": np.ascontiguousarray(bqk_c),
            "wout": np.ascontiguousarray(wout_c),
            "brep": brep_c,
            "trilm": trilm,
            "ones": ones,
        })
    return maps


def kernel(x, W_qkv, b_qkv, W_out, b_out, _trace=False, _trace_kwargs=None):
    x = np.asarray(x, dtype=np.float32)
    W_qkv = np.asarray(W_qkv, dtype=np.float32)
    b_qkv = np.asarray(b_qkv, dtype=np.float32)
    W_out = np.asarray(W_out, dtype=np.float32)
    b_out = np.asarray(b_out, dtype=np.float32)

    nc = _build()
    maps = _in_maps(x, W_qkv, b_qkv, W_out, b_out)
    res = run_bass_kernel_spmd(
        nc, maps, core_ids=list(range(NCORES)), trace=_trace,
        **(_trace_kwargs or {}),
    )
    out = res.results[0]["out_p"]
    for c in range(1, NCORES):
        out = out + res.results[c]["out_p"]
    if _trace:
        _CACHE["last_results"] = res
    return out.astype(np.float32)


# revision 34
# speedup vs baseline: 1.0102x; 1.0102x over previous
"""Causal self-attention (dense transformer block) on 8 TRN2 NeuronCores.

Tensor-parallel over heads: 16 heads / 8 cores -> 2 heads per core, both
batch elements on every core. Per core:
  - QKV projection in "T layout": q^T/k^T per head [dh, tok] (bias fused
    into the PSUM eviction), V natural [tok, dh] for both heads
  - causal attention with scores in transposed layout [k, q], q-chunk
    outer loop (QC=512):
      * scores matmul streams only the causally-live columns (no memsets)
      * softmax numerator via ACT exp (scale folded)
      * denominators accumulate on the PE: ones^T @ P into a PSUM row,
        start/stop spanning the k loop -- no DVE row-sum adds at all
      * unnormalized attn output accumulates in PSUM in [dh, q] layout
      * 1/denominator is broadcast to all partitions with a K=1 matmul
        and applied by DVE at PSUM eviction, so `us` is already
        normalized
  - out_proj per chunk: both heads accumulate into ONE PSUM bank; a
    single DVE tensor_add evicts it and adds the folded bias (v-bias
    term and b_out pre-combined on host)
  - core returns a partial [2, 2048, 2048]; host sums the 8 partials
Matmuls run as float32r (TF32-like, full PE rate at free dim >= 256).
"""
import sys

if "/opt/trn_rl_repo" not in sys.path:
    sys.path.insert(0, "/opt/trn_rl_repo")

import numpy as np

import concourse.bacc as bacc
import concourse.bass as bass
import concourse.mybir as mybir
import concourse.tile as tile
from concourse.bass_utils import run_bass_kernel_spmd

P = 128
B, S, D = 2, 2048, 2048
H, DH = 16, 128
HPC = 2            # heads per core
NCORES = 8
TC = 256           # token chunk for the QKV projection
QC = 512           # q chunk for attention
NQC = S // QC      # 4 q chunks
KPQ = QC // P      # 4 key blocks per q chunk
SCALE = 1.0 / float(np.sqrt(DH))

f32 = mybir.dt.float32
f32r = mybir.dt.float32r
Act = mybir.ActivationFunctionType


def _emit(nc, tc_ctx, aps):
    xty, wqkv, bqk, wout, brep, trilm, ones, out_p = aps
    tc = tc_ctx
    NTB = S // P            # 16 token blocks per batch
    NDC = D // P            # 16 contraction chunks

    with (
        tc.tile_pool(name="const", bufs=1) as const,
        tc.tile_pool(name="xtp", bufs=2) as xtp,
        tc.tile_pool(name="qk", bufs=1) as qk,
        tc.tile_pool(name="vp", bufs=1) as vp,
        tc.tile_pool(name="pp", bufs=4) as pp,
        tc.tile_pool(name="up", bufs=1) as up,
        tc.tile_pool(name="rr", bufs=2) as rr,
        tc.tile_pool(name="rb", bufs=2) as rb,
        tc.tile_pool(name="fin", bufs=4) as fin,
        tc.tile_pool(name="ps_p", bufs=3, space="PSUM") as ps_p,
        tc.tile_pool(name="ps_o", bufs=2, space="PSUM") as ps_o,
        tc.tile_pool(name="ps_u", bufs=2, space="PSUM") as ps_u,
        tc.tile_pool(name="ps_d", bufs=1, space="PSUM") as ps_d,
    ):
        bqk_sb = const.tile([P, 4], f32)
        nc.sync.dma_start(bqk_sb, bqk)
        w_sb = const.tile([P, NDC, 6 * P], f32r)
        wqkv_r = wqkv.rearrange("(dc p) c -> p dc c", p=P).bitcast(f32r)
        for dc in range(NDC):
            nc.sync.dma_start(w_sb[:, dc, :], wqkv_r[:, dc, :])
        tril_sb = const.tile([P, P], f32)
        ones_sb = const.tile([P, 1], f32r)
        onesrow = const.tile([1, P], f32)
        brep_sb = const.tile([P, D], f32)
        wo_sb = const.tile([P, HPC, D], f32r)

        def load_attn_consts():
            nc.sync.dma_start(tril_sb, trilm)
            nc.sync.dma_start(ones_sb, ones.bitcast(f32r))
            nc.sync.dma_start(onesrow, ones.rearrange("p o -> o p"))

        def load_oproj_consts():
            nc.sync.dma_start(
                wo_sb, wout.rearrange("(h p) c -> p h c", p=P).bitcast(f32r)
            )
            nc.sync.dma_start(brep_sb, brep)

        # out-proj tiles are deferred and drip-fed into later loops so the
        # PE queue never blocks on the normalization chain
        pending = []

        def emit_outproj(n):
            for _ in range(min(n, len(pending))):
                b_, tb, cc, us_ = pending.pop(0)
                pso = ps_o.tile([P, QC], f32, tag="o", name="pso")
                nc.tensor.matmul(
                    pso,
                    us_[0][:, tb * P:(tb + 1) * P],
                    wo_sb[:, 0, cc * QC:(cc + 1) * QC],
                    start=True,
                    stop=False,
                )
                nc.tensor.matmul(
                    pso,
                    us_[1][:, tb * P:(tb + 1) * P],
                    wo_sb[:, 1, cc * QC:(cc + 1) * QC],
                    start=False,
                    stop=True,
                )
                f_t = fin.tile([P, QC], f32, tag="fin", name="f_t")
                nc.vector.tensor_add(
                    f_t, pso, brep_sb[:, cc * QC:(cc + 1) * QC]
                )
                nc.sync.dma_start(
                    out_p[b_, tb * P:(tb + 1) * P, cc * QC:(cc + 1) * QC],
                    f_t,
                )

        for b in range(B):
            # ---------------- QKV projection ----------------
            q_sb = [qk.tile([P, S], f32r, tag=f"q{h}", name=f"q{h}") for h in range(HPC)]
            k_sb = [qk.tile([P, S], f32r, tag=f"k{h}", name=f"k{h}") for h in range(HPC)]
            v_sb = vp.tile([P, NTB, HPC * DH], f32r, tag="v", name="v_sb")

            for tci in range(S // TC):
                xt = xtp.tile([P, NDC, TC], f32r, tag="xt", name="xt")
                xsrc = (
                    xty[b, :, tci * TC:(tci + 1) * TC]
                    .rearrange("(dc p) t -> p dc t", p=P)
                    .bitcast(f32r)
                )
                for g in range(4):
                    nc.sync.dma_start(
                        xt[:, 4 * g:4 * g + 4, :], xsrc[:, 4 * g:4 * g + 4, :]
                    )
                # q^T / k^T for both heads: psum [col=128, tok=TC]
                for cb in range(4):
                    psq = (ps_p if cb % 2 else ps_o).tile(
                        [P, TC], f32, tag="o" if cb % 2 == 0 else "w", name="psq"
                    )
                    for dc in range(NDC):
                        nc.tensor.matmul(
                            psq,
                            w_sb[:, dc, cb * P:(cb + 1) * P],
                            xt[:, dc, :],
                            start=(dc == 0),
                            stop=(dc == NDC - 1),
                        )
                    dst = q_sb[cb] if cb < HPC else k_sb[cb - HPC]
                    nc.scalar.activation(
                        dst[:, tci * TC:(tci + 1) * TC],
                        psq,
                        Act.Identity,
                        bias=bqk_sb[:, cb:cb + 1],
                    )
                # V natural for both heads: psum [tok=128, 2*dh]
                for tb in range(TC // P):
                    psv = ps_u.tile([P, HPC * DH], f32, tag="u", name="psv")
                    for dc in range(NDC):
                        nc.tensor.matmul(
                            psv,
                            xt[:, dc, tb * P:(tb + 1) * P],
                            w_sb[:, dc, 4 * P:6 * P],
                            start=(dc == 0),
                            stop=(dc == NDC - 1),
                        )
                    nc.scalar.copy(v_sb[:, tci * (TC // P) + tb, :], psv)
                if b == 0 and tci == 2:
                    load_attn_consts()
                if b == 0 and tci == 4:
                    load_oproj_consts()
                emit_outproj(2)

            # ---------------- attention, q-chunk outer ----------------
            us = [
                up.tile([P, S], f32r, tag=f"u{h}", name=f"u{h}")
                for h in range(HPC)
            ]
            for c in range(NQC):
                nkb = KPQ * c + KPQ          # key blocks 0 .. 4c+3
                psu = [None, None]
                dp = [None, None]
                nunits = max((nkb - 2) * HPC, 1)
                drip = (len(pending) + nunits - 1) // nunits

                # one-deep software pipeline: the PV and denominator matmuls
                # of unit i are emitted AFTER the scores matmul of unit i+1,
                # so the PE always has independent work queued while ACT runs
                # the exp that PV depends on
                def emit_pv(u):
                    kb_, h_, off_, pt_ = u
                    nc.tensor.matmul(
                        psu[h_][:, off_:],
                        v_sb[:, kb_, h_ * DH:(h_ + 1) * DH],
                        pt_[:, off_:],
                        start=(kb_ == 0),
                        stop=(kb_ == nkb - 1),
                    )
                    nc.tensor.matmul(
                        dp[h_][0:1, off_:],
                        ones_sb,
                        pt_[:, off_:],
                        start=(kb_ == 0),
                        stop=(kb_ == nkb - 1),
                    )

                # head-outer runs: only one denominator bank is live at a
                # time (ps_d=1), freeing a bank for the 3-deep scores
                # pipeline; h0's normalization chain overlaps h1's run
                def evict_head(h):
                    # denominators are well away from 0/inf, so the 18-bit
                    # reciprocal is plenty; 5x faster than reciprocal()
                    r = rr.tile([1, QC], f32, tag=f"r{h}", name="rrow")
                    nc.vector.reciprocal_approx_fast(r, dp[h][0:1, :])
                    rp = ps_o.tile([P, QC], f32, tag="o", name="rbc_ps")
                    nc.tensor.matmul(rp, onesrow, r, start=True, stop=True)
                    rbc = rb.tile([P, QC], f32, tag=f"rbc{h}", name="rbc")
                    nc.vector.tensor_copy(out=rbc, in_=rp)
                    nc.vector.tensor_mul(
                        us[h][:, c * QC:(c + 1) * QC],
                        psu[h],
                        rbc,
                    )

                inflight = []
                for h in range(HPC):
                    psu[h] = ps_u.tile([P, QC], f32, tag="u", name=f"psu{h}")
                    dp[h] = ps_d.tile([P, QC], f32, tag="d", name=f"dp{h}")
                    for kb in range(nkb):
                        off = max(0, (kb - KPQ * c) * P)
                        psp = ps_p.tile([P, QC], f32, tag="w", name="psp")
                        nc.tensor.matmul(
                            psp[:, off:],
                            k_sb[h][:, kb * P:(kb + 1) * P],
                            q_sb[h][:, c * QC + off:(c + 1) * QC],
                            start=True,
                            stop=True,
                        )
                        if kb >= KPQ * c:
                            nc.vector.tensor_add(
                                psp[:, off:off + P],
                                psp[:, off:off + P],
                                tril_sb,
                            )
                        p_t = pp.tile([P, QC], f32r, tag="p", name="p_t")
                        nc.scalar.activation(
                            p_t[:, off:], psp[:, off:], Act.Exp, scale=SCALE
                        )
                        # out-proj drip sits between the scores matmul and
                        # the exp-dependent PV pair: independent PE work
                        # covers the ACT latency
                        if kb >= 2:
                            emit_outproj(drip)
                        if len(inflight) == 2:
                            emit_pv(inflight.pop(0))
                        inflight.append((kb, h, off, p_t))
                    if h == 0:
                        # flush h0's tail and start its normalization while
                        # h1's kb-run keeps the PE busy
                        for u in inflight:
                            emit_pv(u)
                        inflight = []
                        evict_head(0)
                for u in inflight:
                    emit_pv(u)
                emit_outproj(len(pending))
                evict_head(1)
                # defer this chunk's out-proj tiles
                for tb in range(KPQ * c, KPQ * c + KPQ):
                    for cc in range(D // QC):
                        pending.append((b, tb, cc, us))
        emit_outproj(len(pending))


_CACHE = {}


def _build():
    if "nc" in _CACHE:
        return _CACHE["nc"]
    nc = bacc.Bacc("TRN2", target_bir_lowering=False, debug=False)
    xty = nc.dram_tensor("xty", [B, D, S], f32, kind="ExternalInput").ap()
    wqkv = nc.dram_tensor("wqkv", [D, 6 * P], f32, kind="ExternalInput").ap()
    bqk = nc.dram_tensor("bqk", [P, 4], f32, kind="ExternalInput").ap()
    wout = nc.dram_tensor("wout", [HPC * DH, D], f32, kind="ExternalInput").ap()
    brep = nc.dram_tensor("brep", [P, D], f32, kind="ExternalInput").ap()
    trilm = nc.dram_tensor("trilm", [P, P], f32, kind="ExternalInput").ap()
    ones = nc.dram_tensor("ones", [P, 1], f32, kind="ExternalInput").ap()
    out_p = nc.dram_tensor("out_p", [B, S, D], f32, kind="ExternalOutput").ap()
    with tile.TileContext(nc) as tctx:
        _emit(nc, tctx, (xty, wqkv, bqk, wout, brep, trilm, ones, out_p))
    nc.compile()
    _CACHE["nc"] = nc
    return nc


def _in_maps(x, W_qkv, b_qkv, W_out, b_out):
    trilm = np.where(
        np.arange(P)[None, :] >= np.arange(P)[:, None], 0.0, -1e9
    ).astype(np.float32)
    ones = np.ones((P, 1), dtype=np.float32)
    xty = np.ascontiguousarray(x.transpose(0, 2, 1))
    maps = []
    for core in range(NCORES):
        h0 = core * HPC
        cols = []
        for off in (0, D, 2 * D):  # q, k, v column groups of W_qkv
            for h in range(h0, h0 + HPC):
                cols.append((off + h * DH, off + (h + 1) * DH))
        wqkv_c = np.concatenate(
            [W_qkv[:, a:b_] for a, b_ in cols], axis=1
        ).astype(np.float32)
        bqk_c = np.stack(
            [b_qkv[a:b_] for a, b_ in cols[:4]], axis=1
        ).astype(np.float32)  # [128, 4]
        bv_c = np.concatenate(
            [b_qkv[a:b_] for a, b_ in cols[4:]]
        ).astype(np.float32)  # [256]
        wout_c = W_out[h0 * DH:(h0 + HPC) * DH, :].astype(np.float32)
        bias_fold = (b_out / NCORES + bv_c @ wout_c).astype(np.float32)
        brep_c = np.broadcast_to(bias_fold, (P, D)).copy()
        maps.append({
            "xty": xty,
            "wqkv": np.ascontiguousarray(wqkv_c),
            "bqk": np.ascontiguousarray(bqk_c),
            "wout": np.ascontiguousarray(wout_c),
            "brep": brep_c,
            "trilm": trilm,
            "ones": ones,
        })
    return maps


def kernel(x, W_qkv, b_qkv, W_out, b_out, _trace=False, _trace_kwargs=None):
    x = np.asarray(x, dtype=np.float32)
    W_qkv = np.asarray(W_qkv, dtype=np.float32)
    b_qkv = np.asarray(b_qkv, dtype=np.float32)
    W_out = np.asarray(W_out, dtype=np.float32)
    b_out = np.asarray(b_out, dtype=np.float32)

    nc = _build()
    maps = _in_maps(x, W_qkv, b_qkv, W_out, b_out)
    res = run_bass_kernel_spmd(
        nc, maps, core_ids=list(range(NCORES)), trace=_trace,
        **(_trace_kwargs or {}),
    )
    out = res.results[0]["out_p"]
    for c in range(1, NCORES):
        out = out + res.results[c]["out_p"]
    if _trace:
        _CACHE["last_results"] = res
    return out.astype(np.float32)
